# revision 62
# baseline (speedup 1.0000x reference)
"""Multi-head self-attention with RoPE (B=2, S=2048, D=1024, H=16, d_k=64,
causal) on 8 trn2 NeuronCores.

Sharding: core c -> batch c//4, heads [4*(c%4), 4*(c%4)+4). Each core gets
x[b]^T, its 4 heads' slices of Wq/Wk/Wv (output dim) and Wo (input dim),
computes a partial y^T = Wo_slice^T . attn_out^T, and the host sums the 4
partials per batch.

Device kernel (per core, bf16 matmuls = 1 PE cycle/row; PSUM stays f32):
  1. QKV projection from x^T (model dim on partitions) producing Q^T/K^T
     (head-d on partitions, 2 heads stacked per 128) and V' (seq on
     partitions). RoPE applied as q*cos + R^T(q*sin), R a signed-permutation
     matmul over a block-of-32 pre-permuted head-d axis.
  2. Transposed-flash attention per (head, 1024-wide q window), k-outer:
     scores^T[k,q] = K^T.T @ Q^T, exp on ACT (scale=1/8), a single DVE
     multiply with a [zeros|triu] mask tile on diagonal k-tiles, then
     attnV out^T[d,q] += V'.T @ P^T accumulated in PSUM. V' carries a ones
     column per head so the softmax denominator accumulates for free.
     Normalization: DVE reciprocal straight off the PSUM denominator row,
     DMA partition-broadcast of the reciprocal, one DVE multiply (odd heads
     add a DMA partition-move into oT rows 64:128). The last window's norm
     is emitted per 512-col half as soon as its accumulation stops.
  3. y^T[o,s] = Wo^T.T @ out^T, staged PSUM->SBUF (DVE/ACT) then one
     batched DMA per 256-row group.

Scheduling: window-0 attention is interleaved with the chunk-2/3
projections, window-1 attention with window-0's output projection, so the
ACT-bound softmax overlaps PE-bound GEMM work instead of serializing.
DMAs are batched into multi-block descriptors (HWDGE descriptor generation
is ~0.6us per DMA regardless of size) and ordered by first consumer so the
PE starts ~3us in. All DMAs ride the SP queue; GpSimd cannot touch PSUM and
SWDGE dispatch stalls the Pool sequencer, so Pool only handles the small
SBUF-to-SBUF ones-column copies.
"""
import os
import sys

import numpy as np

sys.path.insert(0, "/opt/trn_rl_repo")

D_MODEL = 1024
NUM_HEADS = 16
DK = 64
B = 2
S = 2048
THETA = 10000.0
NCORES = 8
HPC = 4          # heads per core
NPAIRS = 2       # head pairs per core
KT = 128         # k tile (partition dim of scores^T)
QW = 1024        # q window
NW = S // QW     # q windows
NI = D_MODEL // 128   # i (contraction) tiles for projections
NCHUNK = S // 512     # 512-wide s chunks

_prog = {}


def _mm_dtype_name():
    return os.environ.get("MHA_MM_DTYPE", "bf16")


def _install_hook_wrapper(bass2jax):
    """Install the neuronx compile hook with a traceback printer (the PJRT
    layer swallows python exceptions from the hook)."""
    import traceback

    bass2jax.install_neuronx_cc_hook()
    import libneuronxla

    if getattr(libneuronxla, "_mha_wrapped", False):
        return
    orig = libneuronxla.neuronx_cc

    def wrapped(*a, **k):
        try:
            return orig(*a, **k)
        except Exception:
            traceback.print_exc()
            raise

    libneuronxla.neuronx_cc = wrapped
    libneuronxla._mha_wrapped = True
    bass2jax.install_neuronx_cc_hook = lambda: None


def _split_excess_waits(nc, max_waits=1):
    """This container's walrus accepts at most one sync-wait per
    instruction; redistribute extras onto same-engine NOPs inserted just
    before the offending instruction."""
    import bass_rust
    import concourse.mybir as mybir

    counter = [0]
    for fn in nc.m.functions:
        for bb in fn.blocks:
            out = []
            changed = False
            for inst in bb.instructions:
                si = inst.sync_info
                waits = list(si.on_wait) if si is not None and si.on_wait else []
                if len(waits) > max_waits:
                    changed = True
                    keep = waits[-max_waits:]
                    extras = waits[:-max_waits]
                    for i in range(0, len(extras), max_waits):
                        counter[0] += 1
                        nop = mybir.InstNoOp(
                            name=f"I-waitsplit-{counter[0]}",
                            ins=[],
                            outs=[],
                            engine=inst.engine,
                        )
                        nop.sync_info = bass_rust.SyncInfo(
                            on_wait=extras[i : i + max_waits], on_update=[]
                        )
                        out.append(nop)
                    si.on_wait = keep
                out.append(inst)
            if changed:
                bb.instructions = out


def _segs(qoff, W=QW):
    """Bank-aligned matmul segments covering [qoff, W). Segments never cross
    a 512-col PSUM bank boundary; for f32r, <256-wide leading segments are
    widened left to 256 (f32r matmuls <256 moving rows cost 4 cycles/row);
    bf16 has no narrow-row penalty so segments stay tight."""
    widen = _mm_dtype_name() == "f32r"
    segs = []
    a = qoff
    while a < W:
        bank_end = (a // 512) * 512 + 512
        b = min(bank_end, W)
        if widen and b - a < 256:
            a2 = max(bank_end - 512, b - 256)
            segs.append((a2, b))
        else:
            segs.append((a, b))
        a = b
    return segs


def _build_program():
    import concourse.bass as bass
    import concourse.mybir as mybir
    from concourse import tile

    F32 = mybir.dt.float32
    MM = {"bf16": mybir.dt.bfloat16, "f32r": mybir.dt.float32r,
          "f32": mybir.dt.float32}[_mm_dtype_name()]
    AF = mybir.ActivationFunctionType
    ALU = mybir.AluOpType

    nc = bass.Bass(target_bir_lowering=False, trn_type="TRN2")

    xt = nc.dram_tensor("xt", [D_MODEL, S], MM, kind="ExternalInput")
    wqt = nc.dram_tensor("wqt", [D_MODEL, 256], MM, kind="ExternalInput")
    wkt = nc.dram_tensor("wkt", [D_MODEL, 256], MM, kind="ExternalInput")
    wvt = nc.dram_tensor("wvt", [D_MODEL, 256], MM, kind="ExternalInput")
    wot = nc.dram_tensor("wot", [256, D_MODEL], MM, kind="ExternalInput")
    cosb = nc.dram_tensor("cosb", [128, S], F32, kind="ExternalInput")
    sinb = nc.dram_tensor("sinb", [128, S], F32, kind="ExternalInput")
    rsign = nc.dram_tensor("rsign", [128, 128], MM, kind="ExternalInput")
    masku = nc.dram_tensor("masku", [128, 256], MM, kind="ExternalInput")
    ones4 = nc.dram_tensor("ones4", [128, 4], MM, kind="ExternalInput")
    yt = nc.dram_tensor("yt", [D_MODEL, S], MM, kind="ExternalOutput")

    with tile.TileContext(nc) as tc:
        with (
            tc.tile_pool(name="const", bufs=1) as cp,
            tc.tile_pool(name="xtp", bufs=5) as xtp,
            tc.tile_pool(name="work", bufs=4) as wk,
            tc.tile_pool(name="norm", bufs=6) as nrm,
            tc.tile_pool(name="yp", bufs=6) as yp,
            tc.tile_pool(name="pT", bufs=6) as pTp,
            tc.tile_pool(name="bc", bufs=4) as bcp,
            tc.tile_pool(name="psA", bufs=2, space="PSUM") as psA,
            tc.tile_pool(name="psB", bufs=2, space="PSUM") as psB,
        ):
            # ---- SBUF residents (DMAs emitted in first-use order below) ----
            w_sb = {
                name: cp.tile([128, NI * 256], MM, tag=f"w{name}", name=f"w{name}")
                for name in ("q", "k", "v")
            }
            wo2_sb = cp.tile([128, NPAIRS * D_MODEL], MM, tag="wo", name="wo")
            wo_sb = [wo2_sb[:, D_MODEL * p : D_MODEL * (p + 1)] for p in range(NPAIRS)]
            cos_sb = cp.tile([128, S], F32, tag="cos")
            sin_sb = cp.tile([128, S], F32, tag="sin")
            r_sb = cp.tile([128, 128], MM, tag="rsign")
            m_sb = cp.tile([128, 256], MM, tag="masku")
            o4_sb = cp.tile([128, 4], MM, tag="ones4")
            qT_sb = [cp.tile([128, S], MM, tag=f"qT{p}", name=f"qT{p}") for p in range(NPAIRS)]
            kT_sb = [cp.tile([128, S], MM, tag=f"kT{p}", name=f"kT{p}") for p in range(NPAIRS)]
            oT_sb = [cp.tile([128, S], MM, tag=f"oT{p}", name=f"oT{p}") for p in range(NPAIRS)]
            v_sb = [cp.tile([128, HPC * 65], MM, tag=f"v{j}", name=f"v{j}") for j in range(S // KT)]

            xs = {}  # chunk -> [128, 4096] x^T tile (8 i-blocks side by side)

            def ld_x(c, i0=0, nblk=NI):
                """DMA i-blocks [i0, i0+nblk) of x^T chunk c into xs[c]."""
                if i0 == 0:
                    xs[c] = xtp.tile([128, NI * 512], MM, tag="xt", name="xt")
                t = xs[c]
                ta = t[:, 0:1]
                nc.sync.dma_start(
                    out=bass.AP(ta.tensor, ta.offset + 512 * i0,
                                [[NI * 512, 128], [512, nblk], [1, 512]]),
                    in_=bass.AP(xt, 2048 * 128 * i0 + 512 * c,
                                [[2048, 128], [128 * 2048, nblk], [1, 512]]),
                )

            def ld_w(name, i0, nblk):
                dram = {"q": wqt, "k": wkt, "v": wvt}[name]
                t = w_sb[name]
                ta = t[:, 0:1]
                nc.sync.dma_start(
                    out=bass.AP(ta.tensor, ta.offset + 256 * i0,
                                [[NI * 256, 128], [256, nblk], [1, 256]]),
                    in_=bass.AP(dram, 256 * 128 * i0,
                                [[256, 128], [128 * 256, nblk], [1, 256]]),
                )

            def ld_sincos(c0, nc_chunks=1):
                sc = slice(512 * c0, 512 * (c0 + nc_chunks))
                nc.sync.dma_start(out=sin_sb[:, sc], in_=sinb[:, sc])
                nc.sync.dma_start(out=cos_sb[:, sc], in_=cosb[:, sc])

            # ---- startup loads, ordered by first consumer ----
            ld_w("q", 0, 2)
            ld_x(0, 0, 2)
            ld_w("q", 2, 3)
            ld_x(0, 2, 3)
            ld_w("q", 5, 3)
            ld_x(0, 5, 3)
            ld_sincos(0, 1)
            nc.sync.dma_start(out=r_sb[:], in_=rsign[:])
            ld_w("k", 0, 4)
            ld_w("k", 4, 4)
            ld_w("v", 0, 8)
            ld_sincos(1, 1)
            nc.sync.dma_start(out=o4_sb[:], in_=ones4[:])
            nc.sync.dma_start(out=m_sb[:], in_=masku[:])
            ld_x(1)
            wa = wo2_sb[:, 0:1]
            nc.sync.dma_start(
                out=bass.AP(wa.tensor, wa.offset,
                            [[NPAIRS * D_MODEL, 128], [D_MODEL, 2], [1, D_MODEL]]),
                in_=bass.AP(wot, 0,
                            [[D_MODEL, 128], [128 * D_MODEL, 2], [1, D_MODEL]]),
            )

            # ---- phase-1 pieces ----
            def proj_qk_mm(c, p, name, st):
                """Piece 1: the 8 accumulation matmuls of one (chunk,pair,q/k)."""
                A = psA.tile([128, QW], F32, tag="A", name="psa")
                st["A"] = A
                ps = A[:, 0:512]
                for i in range(NI):
                    nc.tensor.matmul(
                        out=ps,
                        lhsT=w_sb[name][:, 256 * i + 128 * p : 256 * i + 128 * p + 128],
                        rhs=xs[c][:, 512 * i : 512 * i + 512],
                        start=(i == 0),
                        stop=(i == NI - 1),
                    )

            def proj_qk_rope(c, p, name, st):
                """Piece 2: rope the projected block into qT/kT."""
                sc = slice(512 * c, 512 * c + 512)
                dst = qT_sb if name == "q" else kT_sb
                A = st["A"]
                ps, pssh = A[:, 0:512], A[:, 512:1024]
                tsin = wk.tile([128, 512], MM, tag="tsin", name="tsin")
                nc.vector.tensor_tensor(
                    out=tsin[:], in0=ps, in1=sin_sb[:, sc], op=ALU.mult
                )
                tcos = wk.tile([128, 512], F32, tag="tcos", name="tcos")
                nc.vector.tensor_tensor(
                    out=tcos[:], in0=ps, in1=cos_sb[:, sc], op=ALU.mult
                )
                nc.tensor.matmul(
                    out=pssh, lhsT=r_sb[:], rhs=tsin[:], start=True, stop=True
                )
                nc.vector.tensor_tensor(
                    out=dst[p][:, sc], in0=pssh, in1=tcos[:], op=ALU.add
                )

            def proj_qk(c, p, name):
                st = {}
                proj_qk_mm(c, p, name, st)
                proj_qk_rope(c, p, name, st)

            def proj_v(c, st):
                """Project one 128-row s-tile of V' (4 heads x [ones|dims])."""
                j = 4 * c + st
                stl = slice(128 * st, 128 * st + 128)
                A = psA.tile([128, QW], F32, tag="A", name="psa")
                psv = A[:, 0:256]
                for i in range(NI):
                    nc.tensor.matmul(
                        out=psv,
                        lhsT=xs[c][:, 512 * i + 128 * st : 512 * i + 128 * st + 128],
                        rhs=w_sb["v"][:, 256 * i : 256 * i + 256],
                        start=(i == 0),
                        stop=(i == NI - 1),
                    )
                vt = v_sb[j]
                vb = vt[:, 0:1]
                # per head h: dims at cols 65h..65h+63, ones at col 65h+64
                nc.scalar.activation(
                    out=bass.AP(vb.tensor, vb.offset, [[HPC * 65, 128], [65, 4], [1, 64]]),
                    in_=bass.AP(A.tensor, A.offset, [[QW, 128], [64, 4], [1, 64]]),
                    func=AF.Copy,
                )
                o4 = o4_sb[:, 0:4]
                nc.gpsimd.tensor_copy(
                    out=bass.AP(vb.tensor, vb.offset + 64, [[HPC * 65, 128], [65, 4]]),
                    in_=bass.AP(o4.tensor, o4.offset, [[4, 128], [1, 4]]),
                )

            def proj_chunk_pieces(c):
                for p in range(NPAIRS):
                    yield lambda p=p: proj_qk(c, p, "q")
                for p in range(NPAIRS):
                    yield lambda p=p: proj_qk(c, p, "k")
                for st in range(4):
                    yield lambda st=st: proj_v(c, st)

            # ---- attention ----
            def attn_ktile(w, h, kt, Bt, after_kt=None):
                """scores -> exp -> mask -> attnV for one (head, k-tile)."""
                p, half = divmod(h, 2)
                pr = slice(64 * half, 64 * half + 64)
                vcol = slice(65 * h, 65 * h + 65)
                q0 = QW * w
                k0 = KT * kt
                qoff = max(k0 - q0, 0)
                segs = _segs(qoff)
                A = psA.tile([128, QW], F32, tag="A", name="psa")
                for a, b in segs:
                    nc.tensor.matmul(
                        out=A[:, a:b],
                        lhsT=kT_sb[p][pr, k0 : k0 + KT],
                        rhs=qT_sb[p][pr, q0 + a : q0 + b],
                        start=True,
                        stop=True,
                    )
                pT = pTp.tile([128, QW], MM, tag="pT", name="pT")
                qa = segs[0][0]
                if after_kt is not None:
                    after_kt()
                nc.scalar.activation(
                    out=pT[:, qa:QW], in_=A[:, qa:QW], func=AF.Exp,
                    scale=0.125,
                )
                if k0 >= q0:
                    ext = qoff - qa
                    nc.vector.tensor_tensor(
                        out=pT[:, qa : qoff + KT],
                        in0=pT[:, qa : qoff + KT],
                        in1=m_sb[:, 128 - ext : 256],
                        op=ALU.mult,
                    )
                for a, b in segs:
                    beta = a // 512
                    nc.tensor.matmul(
                        out=Bt[0:65, a:b],
                        lhsT=v_sb[kt][:, vcol],
                        rhs=pT[:, a:b],
                        start=(kt == 0),
                        stop=(kt == (QW // KT) * w + 4 * beta + 3),
                    )

            def attn_head(w, h, after_kt=None, split_norm=False):
                q0 = QW * w
                kmax = (QW // KT) * (w + 1)
                stop0 = (QW // KT) * w + 3  # last kt accumulating cols [0:512)
                p, half = divmod(h, 2)
                Bt = psB.tile([128, QW], F32, tag="B", name="psb")
                for kt in range(kmax):
                    attn_ktile(w, h, kt, Bt, after_kt)
                    if split_norm and kt == stop0:
                        emit_norm(p, half, q0, Bt, Bt[64:65, :], 0, 512)
                if split_norm:
                    emit_norm(p, half, q0, Bt, Bt[64:65, :], 512, QW)
                else:
                    emit_norm(p, half, q0, Bt, Bt[64:65, :], 0, QW)

            def emit_norm(p, half, q0, Bt, dn, a, b):
                """Normalize psum columns [a,b) of one head into oT: recip of
                the free denominator row, DMA partition-broadcast, multiply."""
                wdt = b - a
                rec = nrm.tile([1, QW], F32, tag="rec", name="rec")
                nc.vector.reciprocal(out=rec[:, 0:wdt], in_=dn[:, a:b])
                bct = bcp.tile([64, QW], F32, tag="bc", name="bct")
                bch = bct[0:64, 0:wdt]
                dap = rec[:]
                nc.sync.dma_start(
                    out=bch,
                    in_=bass.AP(dap.tensor, dap.offset, [[QW, 1], [0, 64], [1, wdt]]),
                )
                if half == 0:
                    nc.vector.tensor_tensor(
                        out=oT_sb[p][0:64, q0 + a : q0 + b],
                        in0=Bt[0:64, a:b],
                        in1=bch,
                        op=ALU.mult,
                    )
                else:
                    ot = wk.tile([64, QW], MM, tag="otmp", name="otmp")
                    nc.vector.tensor_tensor(
                        out=ot[:, 0:wdt], in0=Bt[0:64, a:b], in1=bch, op=ALU.mult
                    )
                    nc.sync.dma_start(
                        out=oT_sb[p][64:128, q0 + a : q0 + b], in_=ot[:, 0:wdt]
                    )

            # ---- phase 3: y^T = Wo^T.T @ out^T, straight from PSUM ----
            def p3_group(c, jj, pool, e0, tail=False):
                sc = slice(512 * c, 512 * c + 512)
                P = pool.tile([128, QW], F32, tag="A" if pool is psA else "B", name="p3")
                for half, oc in enumerate((2 * jj, 2 * jj + 1)):
                    ocs = slice(128 * oc, 128 * oc + 128)
                    po = P[:, 512 * half : 512 * half + 512]
                    for p in range(NPAIRS):
                        nc.tensor.matmul(
                            out=po,
                            lhsT=wo_sb[p][:, ocs],
                            rhs=oT_sb[p][:, sc],
                            start=(p == 0),
                            stop=(p == NPAIRS - 1),
                        )
                y_sb = yp.tile([128, QW], MM, tag="y", name="y_sb")
                if tail:
                    nc.vector.tensor_copy(out=y_sb[:, 0:512], in_=P[:, 0:512])
                    nc.scalar.activation(
                        out=y_sb[:, 512:1024], in_=P[:, 512:1024], func=AF.Copy
                    )
                else:
                    nc.vector.tensor_copy(out=y_sb[:], in_=P[:])
                e0.dma_start(
                    out=bass.AP(yt, 2048 * 256 * jj + 512 * c,
                                [[2048, 128], [128 * 2048, 2], [1, 512]]),
                    in_=y_sb[:],
                )

            def run_interleaved(heads, bg, total_kts, split_norm=False,
                                bg_late=None):
                state = {"kt": 0, "emitted": 0, "late": 0}
                late = bg_late or []

                def after_kt():
                    state["kt"] += 1
                    want = state["kt"] * len(bg) // total_kts
                    while state["emitted"] < want:
                        bg[state["emitted"]]()
                        state["emitted"] += 1
                    # late pieces: one per ktile over the final stretch only
                    lwant = max(0, state["kt"] - (total_kts - len(late)))
                    while state["late"] < lwant:
                        late[state["late"]]()
                        state["late"] += 1

                for w, h in heads:
                    attn_head(w, h, after_kt, split_norm=split_norm)
                while state["emitted"] < len(bg):
                    bg[state["emitted"]]()
                    state["emitted"] += 1
                while state["late"] < len(late):
                    late[state["late"]]()
                    state["late"] += 1

            # ---- stage 1: project chunks 0,1 ----
            for piece in proj_chunk_pieces(0):
                piece()
            for piece in proj_chunk_pieces(1):
                piece()

            # ---- stage 2: window-0 attention x chunk-2/3 projections ----
            bg2 = [lambda: ld_x(2), lambda: ld_sincos(2, 2)]
            bg2 += list(proj_chunk_pieces(2))
            bg2 += [lambda: ld_x(3)]
            bg2 += list(proj_chunk_pieces(3))
            run_interleaved([(0, h) for h in (1, 3, 0, 2)], bg2, 4 * (QW // KT))

            # ---- stage 3: window-1 attention x window-0 output proj ----
            bg3 = [
                lambda c=c, jj=jj: p3_group(c, jj, psA, nc.sync)
                for c in (0, 1)
                for jj in range(4)
            ]
            run_interleaved(
                [(1, h) for h in (1, 3, 0, 2)], bg3, 8 * (QW // KT),
                split_norm=True,
            )

            # ---- stage 4: window-1 output proj (tail) ----
            for g, (c, jj) in enumerate((c, jj) for c in (2, 3) for jj in range(4)):
                pool = psA if g % 2 == 0 else psB
                p3_group(c, jj, pool, nc.sync, tail=True)

    _split_excess_waits(nc)
    return nc


def _get_program():
    if "nc" not in _prog:
        from concourse import bass2jax

        _install_hook_wrapper(bass2jax)
        _prog["nc"] = _build_program()
    return _prog["nc"]


def _perm_rows(g):
    """DRAM row order of Wq/Wk for core head-group g: pair-major, head-major,
    evens-then-odds within each head's 64 dims."""
    perm64 = list(range(0, 64, 2)) + list(range(1, 64, 2))
    rows = []
    for h in range(HPC):
        head = HPC * g + h
        rows += [64 * head + j for j in perm64]
    return rows


def _plain_rows(g):
    return [64 * (HPC * g) + j for j in range(64 * HPC)]


def _np_mm_dtype():
    if _mm_dtype_name() == "bf16":
        import ml_dtypes

        return ml_dtypes.bfloat16
    return np.float32


def _host_inputs(x, token_positions, Wq, Wk, Wv, Wo):
    mmdt = _np_mm_dtype()
    x = np.asarray(x, dtype=np.float32)
    pos = np.asarray(token_positions).astype(np.float64)
    Wq = np.asarray(Wq, dtype=np.float32)
    Wk = np.asarray(Wk, dtype=np.float32)
    Wv = np.asarray(Wv, dtype=np.float32)
    Wo = np.asarray(Wo, dtype=np.float32)

    inv = 1.0 / THETA ** (np.arange(0, DK, 2, dtype=np.float64) / DK)
    ang = pos[:, None] * inv[None, :]          # (S, 32)
    cosb = np.tile(np.cos(ang).T.astype(np.float32), (4, 1))  # (128, S)
    sinb = np.tile(np.sin(ang).T.astype(np.float32), (4, 1))

    rsign = np.zeros((128, 128), dtype=np.float32)
    j = np.arange(32)
    for blk in range(2):
        o = 64 * blk
        rsign[o + 32 + j, o + j] = -1.0
        rsign[o + j, o + 32 + j] = 1.0
    masku = np.concatenate([np.zeros((128, 128), np.float32),
                        np.triu(np.ones((128, 128), np.float32))], axis=1)
    ones4 = np.ones((128, 4), dtype=np.float32)

    in_maps = []
    for c in range(NCORES):
        b, g = divmod(c, 4)
        rows = _perm_rows(g)
        vrows = _plain_rows(g)
        in_maps.append(
            {
                "xt": np.ascontiguousarray(x[b].T).astype(mmdt),
                "wqt": np.ascontiguousarray(Wq[rows, :].T).astype(mmdt),
                "wkt": np.ascontiguousarray(Wk[rows, :].T).astype(mmdt),
                "wvt": np.ascontiguousarray(Wv[vrows, :].T).astype(mmdt),
                "wot": np.ascontiguousarray(Wo[:, vrows].T).astype(mmdt),
                "cosb": cosb,
                "sinb": sinb,
                "rsign": rsign.astype(mmdt),
                "masku": masku.astype(mmdt),
                "ones4": ones4.astype(mmdt),
            }
        )
    return in_maps


def run_sharded(x, token_positions, Wq, Wk, Wv, Wo, trace=False):
    from concourse.bass_utils import run_bass_kernel_spmd

    nc = _get_program()
    in_maps = _host_inputs(x, token_positions, Wq, Wk, Wv, Wo)
    res = run_bass_kernel_spmd(
        nc, in_maps, list(range(NCORES)), trace=trace
    )
    y = np.zeros((B, S, D_MODEL), dtype=np.float32)
    for c in range(NCORES):
        y[c // 4] += res.results[c]["yt"].T.astype(np.float32)
    return y, res


def kernel(x, token_positions, Wq, Wk, Wv, Wo):
    y, _ = run_sharded(x, token_positions, Wq, Wk, Wv, Wo)
    return y


def bench_exec(x, token_positions, Wq, Wk, Wv, Wo, iters=5):
    """Steady-state per-call latency of the compiled 8-core executable with
    device-resident inputs (upper bound on HW exec time: includes one axon
    dispatch round-trip)."""
    import time

    import jax
    import concourse.mybir as mybir
    from concourse import bass2jax
    from jax.sharding import Mesh, NamedSharding, PartitionSpec
    from jax.experimental.shard_map import shard_map

    nc = _get_program()
    in_maps = _host_inputs(x, token_positions, Wq, Wk, Wv, Wo)

    partition_name = (
        nc.partition_id_tensor.name if nc.partition_id_tensor else None
    )
    in_names, out_names, out_avals, zero_outs = [], [], [], []
    for alloc in nc.m.functions[0].allocations:
        if not isinstance(alloc, mybir.MemoryLocationSet):
            continue
        name = alloc.memorylocations[0].name
        if alloc.kind == "ExternalInput":
            if name != partition_name:
                in_names.append(name)
        elif alloc.kind == "ExternalOutput":
            shape = tuple(alloc.tensor_shape)
            dtype = mybir.dt.np(alloc.dtype)
            out_names.append(name)
            out_avals.append(jax.core.ShapedArray(shape, dtype))
            zero_outs.append(np.zeros(shape, dtype))
    n_params = len(in_names)
    all_in = in_names + out_names + ([partition_name] if partition_name else [])

    def _body(*args):
        operands = list(args)
        if partition_name is not None:
            operands.append(bass2jax.partition_id_tensor())
        return tuple(
            bass2jax._bass_exec_p.bind(
                *operands,
                out_avals=tuple(out_avals),
                in_names=tuple(all_in),
                out_names=tuple(out_names),
                lowering_input_output_aliases=(),
                sim_require_finite=True,
                sim_require_nnan=True,
                nc=nc,
            )
        )

    devices = jax.devices()[:NCORES]
    mesh = Mesh(np.asarray(devices), ("core",))
    spec = PartitionSpec("core")
    n_in = n_params + len(out_names)
    fn = jax.jit(
        shard_map(
            _body,
            mesh=mesh,
            in_specs=(spec,) * n_in,
            out_specs=(spec,) * len(out_names),
            check_rep=False,
        ),
        keep_unused=True,
    )
    sharding = NamedSharding(mesh, spec)
    args = [
        jax.device_put(
            np.concatenate([np.asarray(in_maps[c][n]) for c in range(NCORES)], 0),
            sharding,
        )
        for n in in_names
    ] + [
        jax.device_put(
            np.zeros((NCORES * z.shape[0], *z.shape[1:]), z.dtype), sharding
        )
        for z in zero_outs
    ]
    out = fn(*args)
    jax.block_until_ready(out)
    t0 = time.time()
    for _ in range(iters):
        out = fn(*args)
        jax.block_until_ready(out)
    per_call = (time.time() - t0) / iters
    return per_call, out
